# revision 7
# baseline (speedup 1.0000x reference)
"""BandSplit (per-band BatchNorm1d + 1x1 Conv1d) on one TRN2 chip (8 NeuronCores).

Sharding: expert-style band parallelism. Each core owns ~4 of the 31 subbands;
each band's BatchNorm (training-mode stats over (B,T)) + 1x1 conv is fully
independent, so there are no cross-core collectives.

Per core the bands are packed into two matmul "groups":
  group0: 2 big bands (K = ciA+ciB <= 50), sections of Kp=64 partitions,
          2 sections (partition bases 0, 64), each section holds 4 batches
          worth of columns.
  group1: 1-2 small bands (K <= 32), sections of Kp=32, 4 sections
          (bases 0/32/64/96), each section holds 2 batches.
Zero-padded partition rows carry zero weights, so they contribute nothing.

On device, BatchNorm is folded into the conv:
    y = (W*diag(s)) @ x + (bias + W^T @ b2)
    s = gamma * rsqrt(var + eps),  b2 = beta - mean * s
Stats come from bn_stats/bn_aggr per partition row; rows of different
sections holding the same channel are combined (and re-broadcast) with one
small PE matmul against a selection matrix.

All data layout packing/unpacking happens on host so every device DMA is a
contiguous [128, 4000] f32 (2 MB) transfer at full port utilization.
"""

import ml_dtypes
import numpy as np

SUBBANDS = [2] + [3] * 10 + [8] * 12 + [16] * 7 + [17]
BAND_START = np.concatenate([[0], np.cumsum(SUBBANDS)[:-1]]).astype(int)
C = 64
B = 8
T = 4000
EPS = 1e-5
NSUB = 500  # matmul free-dim tile (and bn_stats subgroup size)

# per-core band assignment: (group0 bands, group1 bands) — indices into SUBBANDS
CORE_BANDS = [
    ([30, 11], [1, 2]),
    ([23, 12], [3, 4]),
    ([24, 13], [5, 6]),
    ([25, 14], [7, 8]),
    ([26, 15], [9, 10]),
    ([27, 16], [17, 0]),
    ([28, 18], [19, 20]),
    ([29, 21], [22]),
]

GROUP_KP = [64, 32]     # section partition size per group
GROUP_NSEC = [2, 4]     # sections per group
GROUP_NCH = [4, 2]      # input chunks of [128, T] per group

_k = np.arange(128)
SEL = [
    (((_k[:, None] % 64) == (_k[None, :] % 64)).astype(np.float32) * 0.5),
    (((_k[:, None] % 32) == (_k[None, :] % 32)).astype(np.float32) * 0.25),
]

_CACHE = {}


def _build_nc():
    from concourse import bacc, mybir
    import concourse.tile as tile

    f32 = mybir.dt.float32
    bf16 = mybir.dt.bfloat16
    nc = bacc.Bacc("TRN2", target_bir_lowering=False, debug=False, num_devices=8)

    xg = [
        nc.dram_tensor("xg0", [4, 128, T], bf16, kind="ExternalInput"),
        nc.dram_tensor("xg1", [2, 128, T], bf16, kind="ExternalInput"),
    ]
    w_d, sel_d, gam_d, bet_d, bia_d, y_d = [], [], [], [], [], []
    for g in range(2):
        w_d.append(nc.dram_tensor(f"w{g}", [128, 128], bf16, kind="ExternalInput"))
        sel_d.append(nc.dram_tensor(f"sel{g}", [128, 128], f32, kind="ExternalInput"))
        gam_d.append(nc.dram_tensor(f"gamma{g}", [128, 1], f32, kind="ExternalInput"))
        bet_d.append(nc.dram_tensor(f"beta{g}", [128, 1], f32, kind="ExternalInput"))
        bia_d.append(nc.dram_tensor(f"bias{g}", [128, 1], f32, kind="ExternalInput"))
        y_d.append(nc.dram_tensor(f"y{g}", [B, 128, T], bf16, kind="ExternalOutput"))

    with tile.TileContext(nc) as tc, \
         tc.tile_pool(name="xpool", bufs=1) as xpool, \
         tc.tile_pool(name="consts", bufs=1) as consts, \
         tc.tile_pool(name="statsp", bufs=1) as statsp, \
         tc.tile_pool(name="vecs", bufs=1) as vecs, \
         tc.tile_pool(name="wfp", bufs=1) as wfp, \
         tc.tile_pool(name="ostage", bufs=4) as ostage, \
         tc.tile_pool(name="psmm", bufs=5, space="PSUM") as psmm, \
         tc.tile_pool(name="pssm", bufs=2, space="PSUM") as pssm:

        w_t, sel_t, gam_t, bet_t, bia_t = [], [], [], [], []
        for g in range(2):
            wt = consts.tile([128, 128], bf16, tag=f"w{g}")
            nc.sync.dma_start(out=wt[:], in_=w_d[g][:])
            w_t.append(wt)
            st = consts.tile([128, 128], f32, tag=f"sel{g}")
            nc.sync.dma_start(out=st[:], in_=sel_d[g][:])
            sel_t.append(st)
            for lst, dram, nm in ((gam_t, gam_d, "gam"), (bet_t, bet_d, "bet"),
                                  (bia_t, bia_d, "bia")):
                t = consts.tile([128, 1], f32, tag=f"{nm}{g}")
                nc.sync.dma_start(out=t[:], in_=dram[g][:])
                lst.append(t)
        eps_t = consts.tile([128, 1], f32, tag="eps")
        nc.vector.memset(eps_t[:], EPS)

        for g in range(2):
            kp, nsec, nch = GROUP_KP[g], GROUP_NSEC[g], GROUP_NCH[g]

            xcs = []
            for c in range(nch):
                xc = xpool.tile([128, T], bf16, tag=f"x{g}_{c}")
                nc.sync.dma_start(out=xc[:], in_=xg[g][c])
                xcs.append(xc)

            # --- stats: bn_stats per 500-col subgroup, aggregate, combine sections
            stats_t = statsp.tile([128, nch * 8, 6], f32, tag=f"st{g}")
            for c in range(nch):
                for s in range(8):
                    nc.vector.bn_stats(
                        out=stats_t[:, c * 8 + s],
                        in_=xcs[c][:, s * NSUB:(s + 1) * NSUB],
                    )
            mv = vecs.tile([128, 2], f32, tag=f"mv{g}")
            nc.vector.bn_aggr(out=mv[:], in_=stats_t[:])
            # sv = (mean, E[x^2]) per partition row
            sv = vecs.tile([128, 2], f32, tag=f"sv{g}")
            nc.vector.tensor_copy(out=sv[:, 0:1], in_=mv[:, 0:1])
            msq = vecs.tile([128, 1], f32, tag=f"msq{g}")
            nc.vector.tensor_mul(out=msq[:], in0=mv[:, 0:1], in1=mv[:, 0:1])
            nc.vector.tensor_add(out=sv[:, 1:2], in0=mv[:, 1:2], in1=msq[:])
            # combine across sections + broadcast back, via selection matmul
            pst = pssm.tile([128, 2], f32, tag="sm")
            nc.tensor.matmul(pst[:], sel_t[g][:], sv[:], start=True, stop=True)
            est = vecs.tile([128, 2], f32, tag=f"est{g}")
            nc.vector.tensor_copy(out=est[:], in_=pst[:])

            # --- fold BN into conv
            msq2 = vecs.tile([128, 1], f32, tag=f"msq2{g}")
            nc.vector.tensor_mul(out=msq2[:], in0=est[:, 0:1], in1=est[:, 0:1])
            var = vecs.tile([128, 1], f32, tag=f"var{g}")
            nc.vector.tensor_sub(out=var[:], in0=est[:, 1:2], in1=msq2[:])
            std = vecs.tile([128, 1], f32, tag=f"std{g}")
            nc.scalar.activation(out=std[:], in_=var[:],
                                 func=mybir.ActivationFunctionType.Sqrt,
                                 bias=eps_t[:], scale=1.0)
            rstd = vecs.tile([128, 1], f32, tag=f"rstd{g}")
            nc.vector.reciprocal(out=rstd[:], in_=std[:])
            s_t = vecs.tile([128, 1], f32, tag=f"s{g}")
            nc.vector.tensor_mul(out=s_t[:], in0=gam_t[g][:], in1=rstd[:])
            ms = vecs.tile([128, 1], f32, tag=f"ms{g}")
            nc.vector.tensor_mul(out=ms[:], in0=est[:, 0:1], in1=s_t[:])
            b2 = vecs.tile([128, 1], bf16, tag=f"b2{g}")
            nc.vector.tensor_sub(out=b2[:], in0=bet_t[g][:], in1=ms[:])
            wf = wfp.tile([128, 128], bf16, tag=f"wf{g}")
            nc.vector.tensor_scalar_mul(out=wf[:], in0=w_t[g][:], scalar1=s_t[:])
            psb = pssm.tile([128, 1], f32, tag="sm")
            nc.tensor.matmul(psb[:], w_t[g][0:kp, :], b2[0:kp, :],
                             start=True, stop=True)
            bf = vecs.tile([128, 1], f32, tag=f"bf{g}")
            nc.vector.tensor_add(out=bf[:], in0=psb[:], in1=bia_t[g][:])

            # --- main matmuls + bias epilogue + output DMA
            for c in range(nch):
                for q in range(nsec):
                    base = kp * q
                    b_idx = nch * q + c
                    stage = ostage.tile([128, T], bf16, tag="stage")
                    for u in range(8):
                        ps = psmm.tile([128, NSUB], f32, tag="mm")
                        nc.tensor.matmul(
                            ps[:],
                            wf[base:base + kp, :],
                            xcs[c][base:base + kp, u * NSUB:(u + 1) * NSUB],
                            start=True, stop=True,
                            tile_position=(base, 0),
                        )
                        if u % 2 == 0:
                            nc.scalar.add(out=stage[:, u * NSUB:(u + 1) * NSUB],
                                          in_=ps[:], add=bf[:])
                        else:
                            nc.vector.tensor_scalar_add(
                                out=stage[:, u * NSUB:(u + 1) * NSUB],
                                in0=ps[:], scalar1=bf[:])
                    nc.scalar.dma_start(out=y_d[g][b_idx], in_=stage[:])

    nc.compile()
    return nc


def _band_x(spec, i):
    s, sb = BAND_START[i], SUBBANDS[i]
    return spec[:, s:s + sb].reshape(B, 2 * sb, T)


def _make_in_maps(spec, weights, biases, gammas, betas):
    in_maps = []
    for core in range(8):
        im = {}
        for g, bands in enumerate(CORE_BANDS[core]):
            kp, nsec, nch = GROUP_KP[g], GROUP_NSEC[g], GROUP_NCH[g]
            xcat = np.concatenate([_band_x(spec, i) for i in bands], axis=1)
            K = xcat.shape[1]
            xgh = np.zeros((nch, 128, T), np.float32)
            for q in range(nsec):
                # section q (partitions kp*q..kp*q+K), chunk c -> batch nch*q+c
                xgh[:, kp * q:kp * q + K, :] = xcat[nch * q:nch * (q + 1)]
            im[f"xg{g}"] = xgh.astype(ml_dtypes.bfloat16)

            blk = np.zeros((kp, 128), np.float32)
            gam = np.zeros((kp,), np.float32)
            bet = np.zeros((kp,), np.float32)
            biasv = np.zeros((128, 1), np.float32)
            off = 0
            for bi, band in enumerate(bands):
                ci = 2 * SUBBANDS[band]
                blk[off:off + ci, 64 * bi:64 * bi + 64] = \
                    np.asarray(weights[band], np.float32).T
                gam[off:off + ci] = np.asarray(gammas[band], np.float32)
                bet[off:off + ci] = np.asarray(betas[band], np.float32)
                biasv[64 * bi:64 * bi + 64, 0] = np.asarray(biases[band], np.float32)
                off += ci
            w = np.zeros((128, 128), np.float32)
            gmv = np.zeros((128, 1), np.float32)
            btv = np.zeros((128, 1), np.float32)
            for q in range(128 // kp):
                w[kp * q:kp * (q + 1)] = blk
                gmv[kp * q:kp * q + kp, 0] = gam
                btv[kp * q:kp * q + kp, 0] = bet
            im[f"w{g}"] = w.astype(ml_dtypes.bfloat16)
            im[f"gamma{g}"] = gmv
            im[f"beta{g}"] = btv
            im[f"bias{g}"] = biasv
            im[f"sel{g}"] = SEL[g]
        in_maps.append(im)
    return in_maps


def _gather(results):
    out = np.empty((B, 31, C, T), np.float32)
    for core in range(8):
        for g, bands in enumerate(CORE_BANDS[core]):
            yg = np.asarray(results[core][f"y{g}"]).astype(np.float32)
            for bi, band in enumerate(bands):
                out[:, band] = yg[:, 64 * bi:64 * bi + 64]
    return out


def run(trace=False, trace_cores=None, **inputs):
    from concourse.bass_utils import run_bass_kernel_spmd

    spec = np.ascontiguousarray(np.asarray(inputs["spec_noisy"], np.float32))
    in_maps = _make_in_maps(spec, inputs["weights"], inputs["biases"],
                            inputs["gammas"], inputs["betas"])
    if "nc" not in _CACHE:
        _CACHE["nc"] = _build_nc()
    nc = _CACHE["nc"]
    res = run_bass_kernel_spmd(nc, in_maps, core_ids=list(range(8)),
                               trace=trace, trace_cores=trace_cores)
    return _gather(res.results), res


def kernel(**inputs):
    out, _ = run(trace=False, **inputs)
    return out


# revision 14
# speedup vs baseline: 1.0171x; 1.0171x over previous
"""BandSplit (per-band BatchNorm1d + 1x1 Conv1d) on one TRN2 chip (8 NeuronCores).

Sharding: expert-style band parallelism. Each core owns ~4 of the 31 subbands;
each band's BatchNorm (training-mode stats over (B,T)) + 1x1 conv is fully
independent, so there are no cross-core collectives.

Per core the bands are packed into two matmul "groups":
  group0: 2 big bands (K = ciA+ciB <= 50), sections of Kp=64 partitions,
          2 sections (partition bases 0, 64), each section holds 4 batches
          worth of columns.
  group1: 1-2 small bands (K <= 32), sections of Kp=32, 4 sections
          (bases 0/32/64/96), each section holds 2 batches.
Zero-padded partition rows carry zero weights, so they contribute nothing.

On device, BatchNorm is folded into the conv:
    y = (W*diag(s)) @ x + (bias + W^T @ b2)
    s = gamma * rsqrt(var + eps),  b2 = beta - mean * s
Stats come from bn_stats/bn_aggr per partition row; rows of different
sections holding the same channel are combined (and re-broadcast) with one
small PE matmul against a selection matrix.

All data layout packing/unpacking happens on host so every device DMA is a
contiguous [128, 4000] f32 (2 MB) transfer at full port utilization.
"""

import ml_dtypes
import numpy as np

SUBBANDS = [2] + [3] * 10 + [8] * 12 + [16] * 7 + [17]
BAND_START = np.concatenate([[0], np.cumsum(SUBBANDS)[:-1]]).astype(int)
C = 64
B = 8
T = 4000
EPS = 1e-5
NSUB = 500  # matmul free-dim tile (and bn_stats subgroup size)

# per-core band assignment: (group0 bands, group1 bands) — indices into SUBBANDS
CORE_BANDS = [
    ([30, 11], [1, 2]),
    ([23, 12], [3, 4]),
    ([24, 13], [5, 6]),
    ([25, 14], [7, 8]),
    ([26, 15], [9, 10]),
    ([27, 16], [17, 0]),
    ([28, 18], [19, 20]),
    ([29, 21], [22]),
]

GROUP_KP = [64, 32]     # section partition size per group
GROUP_NSEC = [2, 4]     # sections per group
GROUP_NCH = [4, 2]      # input chunks of [128, T] per group

# selection matrices fold the full-count normalization (each channel sees
# B*T = 32000 elements across its sections), so sel @ (sum, sqsum) = (mean, E2)
_k = np.arange(128)
SEL = [
    (((_k[:, None] % 64) == (_k[None, :] % 64)).astype(np.float32) / 32000.0),
    (((_k[:, None] % 32) == (_k[None, :] % 32)).astype(np.float32) / 32000.0),
]

_CACHE = {}


def _build_nc():
    from concourse import bacc, mybir
    import concourse.tile as tile

    f32 = mybir.dt.float32
    bf16 = mybir.dt.bfloat16
    nc = bacc.Bacc("TRN2", target_bir_lowering=False, debug=False, num_devices=8)

    xg = [
        nc.dram_tensor("xg0", [4, 128, T], bf16, kind="ExternalInput"),
        nc.dram_tensor("xg1", [2, 128, T], bf16, kind="ExternalInput"),
    ]
    w_d, sel_d, gam_d, bet_d, bia_d, y_d = [], [], [], [], [], []
    for g in range(2):
        w_d.append(nc.dram_tensor(f"w{g}", [128, 128], bf16, kind="ExternalInput"))
        sel_d.append(nc.dram_tensor(f"sel{g}", [128, 128], f32, kind="ExternalInput"))
        gam_d.append(nc.dram_tensor(f"gamma{g}", [128, 1], f32, kind="ExternalInput"))
        bet_d.append(nc.dram_tensor(f"beta{g}", [128, 1], f32, kind="ExternalInput"))
        bia_d.append(nc.dram_tensor(f"bias{g}", [128, 1], f32, kind="ExternalInput"))
        y_d.append(nc.dram_tensor(f"y{g}", [B, 128, T], bf16, kind="ExternalOutput"))

    with tile.TileContext(nc) as tc, \
         tc.tile_pool(name="xpool", bufs=1) as xpool, \
         tc.tile_pool(name="consts", bufs=1) as consts, \
         tc.tile_pool(name="statsp", bufs=1) as statsp, \
         tc.tile_pool(name="vecs", bufs=1) as vecs, \
         tc.tile_pool(name="wfp", bufs=1) as wfp, \
         tc.tile_pool(name="ostage", bufs=4) as ostage, \
         tc.tile_pool(name="psmm", bufs=3, space="PSUM") as psmm, \
         tc.tile_pool(name="pssm", bufs=2, space="PSUM") as pssm:

        w_t, sel_t, gam_t, bet_t, bia_t = [], [], [], [], []
        for g in range(2):
            wt = consts.tile([128, 128], bf16, tag=f"w{g}")
            nc.sync.dma_start(out=wt[:], in_=w_d[g][:])
            w_t.append(wt)
            st = consts.tile([128, 128], f32, tag=f"sel{g}")
            nc.sync.dma_start(out=st[:], in_=sel_d[g][:])
            sel_t.append(st)
            for lst, dram, nm in ((gam_t, gam_d, "gam"), (bet_t, bet_d, "bet"),
                                  (bia_t, bia_d, "bia")):
                t = consts.tile([128, 1], f32, tag=f"{nm}{g}")
                nc.sync.dma_start(out=t[:], in_=dram[g][:])
                lst.append(t)
        eps_t = consts.tile([128, 1], f32, tag="eps")
        nc.vector.memset(eps_t[:], EPS)
        self_evac_idx = [0]

        for g in range(2):
            kp, nsec, nch = GROUP_KP[g], GROUP_NSEC[g], GROUP_NCH[g]

            xcs = []
            for c in range(nch):
                xc = xpool.tile([128, T], bf16, tag=f"x{g}_{c}")
                nc.sync.dma_start(out=xc[:], in_=xg[g][c])
                xcs.append(xc)

            # --- stats: per-chunk per-row sum and sum-of-squares on DVE; the
            # 1/32000 normalization is folded into the selection matrix, so the
            # sel matmul directly yields (mean, E[x^2]) broadcast to all rows
            sums = statsp.tile([128, 2, nch], f32, tag=f"sums{g}")
            for c in range(nch):
                scr_s = statsp.tile([128, T], bf16, tag="scr_s", bufs=2)
                nc.vector.tensor_scalar(
                    out=scr_s[:], in0=xcs[c][:], scalar1=1.0, scalar2=None,
                    op0=mybir.AluOpType.mult, op1=mybir.AluOpType.add,
                    accum_out=sums[:, 0, c:c + 1])
                scr_v = statsp.tile([128, T], bf16, tag="scr_v", bufs=2)
                nc.vector.scalar_tensor_tensor(
                    out=scr_v[:], in0=xcs[c][:], scalar=0.0, in1=xcs[c][:],
                    op0=mybir.AluOpType.add, op1=mybir.AluOpType.mult,
                    accum_out=sums[:, 1, c:c + 1])
            # sv = per-row (sum, sqsum)
            sv = vecs.tile([128, 2], f32, tag=f"sv{g}")
            nc.vector.tensor_reduce(out=sv[:], in_=sums[:], op=mybir.AluOpType.add,
                                    axis=mybir.AxisListType.X)
            # combine across sections + broadcast back, via selection matmul
            pst = pssm.tile([128, 2], f32, tag="sm")
            nc.tensor.matmul(pst[:], sel_t[g][:], sv[:], start=True, stop=True)
            est = vecs.tile([128, 2], f32, tag=f"est{g}")
            nc.vector.tensor_copy(out=est[:], in_=pst[:])

            # --- fold BN into conv
            msq2 = vecs.tile([128, 1], f32, tag=f"msq2{g}")
            nc.vector.tensor_mul(out=msq2[:], in0=est[:, 0:1], in1=est[:, 0:1])
            var = vecs.tile([128, 1], f32, tag=f"var{g}")
            nc.vector.tensor_sub(out=var[:], in0=est[:, 1:2], in1=msq2[:])
            std = vecs.tile([128, 1], f32, tag=f"std{g}")
            nc.scalar.activation(out=std[:], in_=var[:],
                                 func=mybir.ActivationFunctionType.Sqrt,
                                 bias=eps_t[:], scale=1.0)
            rstd = vecs.tile([128, 1], f32, tag=f"rstd{g}")
            nc.vector.reciprocal(out=rstd[:], in_=std[:])
            s_t = vecs.tile([128, 1], f32, tag=f"s{g}")
            nc.vector.tensor_mul(out=s_t[:], in0=gam_t[g][:], in1=rstd[:])
            ms = vecs.tile([128, 1], f32, tag=f"ms{g}")
            nc.vector.tensor_mul(out=ms[:], in0=est[:, 0:1], in1=s_t[:])
            b2 = vecs.tile([128, 1], bf16, tag=f"b2{g}")
            nc.vector.tensor_sub(out=b2[:], in0=bet_t[g][:], in1=ms[:])
            wf = wfp.tile([128, 128], bf16, tag=f"wf{g}")
            nc.vector.tensor_scalar_mul(out=wf[:], in0=w_t[g][:], scalar1=s_t[:])
            psb = pssm.tile([128, 1], f32, tag="sm")
            nc.tensor.matmul(psb[:], w_t[g][0:kp, :], b2[0:kp, :],
                             start=True, stop=True)
            bf = vecs.tile([128, 1], f32, tag=f"bf{g}")
            nc.vector.tensor_add(out=bf[:], in0=psb[:], in1=bia_t[g][:])

            # --- main matmuls + bias epilogue + output DMA
            # two matmuls per 2-bank psum tile (cols 0 and 512, 500 wide each),
            # evacuated by one DVE/ACT op over a [128, 2, 500] AP
            for c in range(nch):
                for q in range(nsec):
                    base = kp * q
                    b_idx = nch * q + c
                    stage = ostage.tile([128, T], bf16, tag="stage")
                    for u2 in range(4):
                        ps = psmm.tile([128, 1024], f32, tag="mm")
                        for h in range(2):
                            u = u2 * 2 + h
                            nc.tensor.matmul(
                                ps[:, 512 * h:512 * h + NSUB],
                                wf[base:base + kp, :],
                                xcs[c][base:base + kp, u * NSUB:(u + 1) * NSUB],
                                start=True, stop=True,
                                tile_position=(base, 0),
                            )
                        pv = ps[:].rearrange("p (a b) -> p a b", a=2)[:, :, 0:NSUB]
                        so = stage[:, u2 * 1000:(u2 + 1) * 1000].rearrange(
                            "p (a b) -> p a b", a=2)
                        if self_evac_idx[0] % 8 in (0, 3, 5):
                            nc.vector.tensor_scalar_add(out=so, in0=pv,
                                                        scalar1=bf[:])
                        else:
                            nc.scalar.add(out=so, in_=pv, add=bf[:])
                        self_evac_idx[0] += 1
                    nc.gpsimd.dma_start(out=y_d[g][b_idx], in_=stage[:])

    nc.compile()
    return nc


def _band_x(spec, i):
    s, sb = BAND_START[i], SUBBANDS[i]
    return spec[:, s:s + sb].reshape(B, 2 * sb, T)


def _make_in_maps(spec, weights, biases, gammas, betas):
    in_maps = []
    for core in range(8):
        im = {}
        for g, bands in enumerate(CORE_BANDS[core]):
            kp, nsec, nch = GROUP_KP[g], GROUP_NSEC[g], GROUP_NCH[g]
            xcat = np.concatenate([_band_x(spec, i) for i in bands], axis=1)
            K = xcat.shape[1]
            xgh = np.zeros((nch, 128, T), np.float32)
            for q in range(nsec):
                # section q (partitions kp*q..kp*q+K), chunk c -> batch nch*q+c
                xgh[:, kp * q:kp * q + K, :] = xcat[nch * q:nch * (q + 1)]
            im[f"xg{g}"] = xgh.astype(ml_dtypes.bfloat16)

            blk = np.zeros((kp, 128), np.float32)
            gam = np.zeros((kp,), np.float32)
            bet = np.zeros((kp,), np.float32)
            biasv = np.zeros((128, 1), np.float32)
            off = 0
            for bi, band in enumerate(bands):
                ci = 2 * SUBBANDS[band]
                blk[off:off + ci, 64 * bi:64 * bi + 64] = \
                    np.asarray(weights[band], np.float32).T
                gam[off:off + ci] = np.asarray(gammas[band], np.float32)
                bet[off:off + ci] = np.asarray(betas[band], np.float32)
                biasv[64 * bi:64 * bi + 64, 0] = np.asarray(biases[band], np.float32)
                off += ci
            w = np.zeros((128, 128), np.float32)
            gmv = np.zeros((128, 1), np.float32)
            btv = np.zeros((128, 1), np.float32)
            for q in range(128 // kp):
                w[kp * q:kp * (q + 1)] = blk
                gmv[kp * q:kp * q + kp, 0] = gam
                btv[kp * q:kp * q + kp, 0] = bet
            im[f"w{g}"] = w.astype(ml_dtypes.bfloat16)
            im[f"gamma{g}"] = gmv
            im[f"beta{g}"] = btv
            im[f"bias{g}"] = biasv
            im[f"sel{g}"] = SEL[g]
        in_maps.append(im)
    return in_maps


def _gather(results):
    out = np.empty((B, 31, C, T), np.float32)
    for core in range(8):
        for g, bands in enumerate(CORE_BANDS[core]):
            yg = np.asarray(results[core][f"y{g}"]).astype(np.float32)
            for bi, band in enumerate(bands):
                out[:, band] = yg[:, 64 * bi:64 * bi + 64]
    return out


def run(trace=False, trace_cores=None, **inputs):
    from concourse.bass_utils import run_bass_kernel_spmd

    spec = np.ascontiguousarray(np.asarray(inputs["spec_noisy"], np.float32))
    in_maps = _make_in_maps(spec, inputs["weights"], inputs["biases"],
                            inputs["gammas"], inputs["betas"])
    if "nc" not in _CACHE:
        _CACHE["nc"] = _build_nc()
    nc = _CACHE["nc"]
    res = run_bass_kernel_spmd(nc, in_maps, core_ids=list(range(8)),
                               trace=trace, trace_cores=trace_cores)
    return _gather(res.results), res


def kernel(**inputs):
    out, _ = run(trace=False, **inputs)
    return out
